# revision 6
# baseline (speedup 1.0000x reference)
"""Expert-parallel MoE kernel for Trainium2 (8 NeuronCores).

Strategy (expert-parallel, per sharding hint):
  - Host: sort the T*top_k dispatch pairs by expert, scale each dispatched
    token by its gate score (gate folds into the linear map's input), pad
    each expert's token group to a fixed capacity CAP, lay out chunk-major
    (pre-transposed for the PE's lhsT operand), cast to bf16.
  - Device (SPMD, core c owns experts 2c and 2c+1): Z_e = X_e^T.T @ W_e
    as tiled bf16 matmuls with fp32 PSUM accumulation.
      * all loads ride one HWDGE ring (sync) in k-consumption order so the
        PE is fed continuously; chunks shrink toward the end so the last
        matmul's data dependency is small
      * dummy warm-up matmuls keep the PE HAM clock-gate busy from t=0 so
        real matmuls run at 2.4 GHz
      * PSUM->SBUF copies split across DVE (n0) and ACT (n1) in parallel
      * stores queue on the sync ring BEHIND all loads (FIFO keeps store
        traffic out of the load window); the final store goes on the
        scalar ring so the two tails drain in parallel
  - Host: scatter Z rows back to dispatch pairs, sum top_k contributions,
    add the (gate-weighted) expert biases.
"""

import numpy as np
import ml_dtypes

NUM_EXPERT = 16
D = 1024
TOP_K = 2
T = 2048
N_CORES = 8
EPC = NUM_EXPERT // N_CORES  # experts per core
CAP = 256                    # per-expert dispatch capacity (multiple of 128)
KT = D // 128                # contraction tiles (8)
NT = D // 512                # output free-dim tiles (one PSUM bank each)
MT = CAP // 128              # token tiles (2)

N_DUMMY = 16                 # PE warm-up matmuls (HAM clock-gate)

# per-expert load chunks, in PE consumption order.  Each chunk is a
# [128, f] tile whose per-partition data is contiguous in DRAM.
#   x   : [128p, KT*CAP]  (p = k index within k-tile)
#   Wk  : k-tiles of W, [128p, kl*D]
#   k7 is split by n so the final matmuls' data dependencies are small.
X_ELEMS = KT * 128 * CAP              # 262144
W_ELEMS = KT * 128 * D                # 1048576
# (kind, k0, klen, n0, nlen) — kind 'x' or 'w'
CHUNKS = [
    ("x", 0, 0, 0, 0),
    ("w", 0, 2, 0, D),
    ("w", 2, 2, 0, D),
    ("w", 4, 2, 0, D),
    ("w", 6, 1, 0, D),
    ("w", 7, 1, 0, 512),
    ("w", 7, 1, 512, 512),
]
E_ELEMS = X_ELEMS + W_ELEMS

TRACE = False                # set by test harness to collect an NTFF profile
LAST_RESULT = None           # BassKernelResults of the most recent run

_NC = None


def _build_nc():
    from concourse import bacc, tile
    import concourse.mybir as mybir

    bf16 = mybir.dt.bfloat16
    f32 = mybir.dt.float32

    nc = bacc.Bacc("TRN2", target_bir_lowering=False, debug=False,
                   num_devices=N_CORES)
    a = nc.declare_dram_parameter("a", [EPC, E_ELEMS], bf16, isOutput=False)
    z = nc.declare_dram_parameter("z", [EPC, CAP, D], bf16, isOutput=True)

    with tile.TileContext(nc, num_cores=N_CORES) as tc:
        with (
            tc.tile_pool(name="wp", bufs=1) as wp,
            tc.tile_pool(name="sp", bufs=1) as sp,
            tc.tile_pool(name="pp", bufs=2, space="PSUM") as pp,
            tc.tile_pool(name="op", bufs=4) as op,
        ):
            # --- PE warm-up: tiny independent matmuls on a scratch tile
            # keep the PE HAM activity monitor busy from t~0.5us so the
            # clock gate opens (1.2 -> 2.4 GHz) before real data arrives.
            scr = sp.tile([128, 64], bf16, name="scr", tag="scr")
            nc.gpsimd.memset(scr[:], 0.0)
            # dummy PSUM tile shares tag "ps11" buffer rotation: dummy and
            # expert-1's ps11 use the same bank (WAW ordered, e1 starts
            # late so the dummies never delay it).
            psd = pp.tile([128, 512], f32, name="psd", tag="ps11")
            for i in range(N_DUMMY):
                nc.tensor.matmul(psd[:64, :64], scr[:, :64], scr[:, :64],
                                 start=True, stop=True)

            # --- all load DMAs on the sync HWDGE ring, in k order
            xts, wts = {}, {}
            for e in range(EPC):
                for (kind, k0, kl, n0, nl) in CHUNKS:
                    if kind == "x":
                        t_ = wp.tile([128, KT * CAP], bf16,
                                     name=f"x{e}", tag=f"x{e}")
                        src = a[e][0:X_ELEMS]
                        nc.sync.dma_start(
                            t_[:], src.rearrange("(p f) -> p f", p=128))
                        xts[e] = t_
                    else:
                        t_ = wp.tile([128, kl * nl], bf16,
                                     name=f"w{e}_{k0}_{n0}",
                                     tag=f"w{e}_{k0}_{n0}")
                        if nl == D:
                            off = X_ELEMS + k0 * 128 * D
                            src = a[e][off:off + kl * 128 * D]
                        else:
                            # n-split chunk of one k-tile: host lays the two
                            # halves consecutively
                            off = X_ELEMS + k0 * 128 * D + (n0 // 512) * 128 * 512
                            src = a[e][off:off + 128 * 512]
                        nc.sync.dma_start(
                            t_[:], src.rearrange("(p f) -> p f", p=128))
                        if nl == D:
                            for kk in range(kl):
                                for n in range(NT):
                                    wts[e, k0 + kk, n] = (t_, kk * D + n * 512)
                        else:
                            wts[e, k0, n0 // 512] = (t_, 0)

            # --- matmuls, k-outer per expert; 4 (m,n) PSUM banks per
            # expert accumulate in parallel; experts double-buffer banks
            for e in range(EPC):
                pss = {}
                for m in range(MT):
                    for n in range(NT):
                        pss[m, n] = pp.tile([128, 512], f32,
                                            name=f"ps{e}_{m}{n}",
                                            tag=f"ps{m}{n}")
                xt = xts[e]
                for k in range(KT):
                    for n in range(NT):
                        wtl, noff = wts[e, k, n]
                        for m in range(MT):
                            nc.tensor.matmul(
                                pss[m, n][:],
                                xt[:, k * CAP + m * 128:
                                   k * CAP + (m + 1) * 128],
                                wtl[:, noff:noff + 512],
                                start=(k == 0),
                                stop=(k == KT - 1),
                            )
                # copies: n0 on DVE, n1 on ACT (parallel); stores queue on
                # the sync ring behind all loads except the very last one,
                # which goes on the scalar ring so both tails overlap.
                for m in range(MT):
                    ot = op.tile([128, D], bf16, name=f"o{e}_{m}",
                                 tag=f"o{e}_{m}")
                    nc.vector.tensor_copy(ot[:, 0:512], pss[m, 0][:])
                    nc.scalar.copy(ot[:, 512:D], pss[m, 1][:])
                    eng = nc.scalar if (e, m) == (EPC - 1, MT - 1) else nc.sync
                    eng.dma_start(z[e, m * 128:(m + 1) * 128, :], ot[:])
    nc.compile()
    return nc


def _pack_inputs(inp, gi, gs, W):
    """Sort dispatch pairs by expert, gate-fold, pad to CAP, and lay out
    the per-core DRAM image in device chunk order.  Returns (a_dev, sel,
    rnk, overflow, fe, tok, fg)."""
    P = T * TOP_K
    fe = gi.reshape(P)
    fg = gs.reshape(P)
    tok = np.arange(P) // TOP_K

    order = np.argsort(fe, kind="stable")
    counts = np.bincount(fe, minlength=NUM_EXPERT)
    starts = np.zeros(NUM_EXPERT + 1, np.int64)
    np.cumsum(counts, out=starts[1:])
    rank = np.arange(P) - starts[fe[order]]
    ok = rank < CAP
    sel = order[ok]
    rnk = rank[ok]

    xpad = np.zeros((NUM_EXPERT, CAP, D), np.float32)
    xpad[fe[sel], rnk] = inp[tok[sel]] * fg[sel, None]

    # x chunk: [E, 128p, KT, CAP] -> flat
    xk = xpad.reshape(NUM_EXPERT, CAP, KT, 128)
    x_dev = xk.transpose(0, 3, 2, 1).astype(ml_dtypes.bfloat16) \
              .reshape(NUM_EXPERT, X_ELEMS)
    # W: k-major [E, KT, 128, D]; k7 n-split halves are already consecutive
    # when flattened ([128, 512] blocks) -- handle explicitly:
    wk = W.reshape(NUM_EXPERT, KT, 128, D).astype(ml_dtypes.bfloat16)
    # each multi-k chunk is tile-layout [128p, kl, D] -> per-chunk transpose
    w_parts = [
        wk[:, k0:k0 + 2].transpose(0, 2, 1, 3).reshape(NUM_EXPERT, -1)
        for k0 in (0, 2, 4)
    ]
    w_parts.append(wk[:, 6].reshape(NUM_EXPERT, 128 * D))
    k7 = wk[:, 7]                                   # [E, 128, D]
    w_parts.append(np.ascontiguousarray(k7[:, :, 0:512]).reshape(NUM_EXPERT, -1))
    w_parts.append(np.ascontiguousarray(k7[:, :, 512:D]).reshape(NUM_EXPERT, -1))
    a_dev = np.concatenate([x_dev] + w_parts, axis=1)
    assert a_dev.shape[1] == E_ELEMS
    return a_dev, sel, rnk, order[~ok], fe, tok, fg


def kernel(inp, gate_idx, gate_score, W, b):
    global _NC, LAST_RESULT
    from concourse.bass_utils import run_bass_kernel_spmd

    inp = np.ascontiguousarray(np.asarray(inp, dtype=np.float32))
    gi = np.asarray(gate_idx).astype(np.int64)
    gs = np.asarray(gate_score, dtype=np.float32)
    W = np.asarray(W, dtype=np.float32)
    b = np.asarray(b, dtype=np.float32)

    a_dev, sel, rnk, overflow, fe, tok, fg = _pack_inputs(inp, gi, gs, W)

    if _NC is None:
        _NC = _build_nc()

    in_maps = [
        {"a": a_dev[c * EPC:(c + 1) * EPC]}
        for c in range(N_CORES)
    ]
    res = run_bass_kernel_spmd(_NC, in_maps, list(range(N_CORES)),
                               trace=TRACE)
    LAST_RESULT = res
    zall = np.concatenate(
        [np.asarray(r["z"]).astype(np.float32) for r in res.results],
        axis=0)  # [E,CAP,D]

    P = T * TOP_K
    zpairs = np.zeros((P, D), np.float32)
    zpairs[sel] = zall[fe[sel], rnk]
    # exact f32 fallback for over-capacity pairs (~2% of dispatches)
    if overflow.size:
        fe_o = fe[overflow]
        for e in np.unique(fe_o):
            pi = overflow[fe_o == e]
            zpairs[pi] = (inp[tok[pi]] * fg[pi, None]) @ W[e]

    y = zpairs.reshape(T, TOP_K, D).sum(axis=1)
    y += (gs[:, :, None] * b[gi]).sum(axis=1)
    return y.astype(np.float32)


# revision 7
# speedup vs baseline: 1.0941x; 1.0941x over previous
"""Expert-parallel MoE kernel for Trainium2 (8 NeuronCores).

Strategy (expert-parallel, per sharding hint):
  - Host: sort the T*top_k dispatch pairs by expert, scale each dispatched
    token by its gate score (gate folds into the linear map's input), pad
    each expert's token group to a fixed capacity CAP, lay out chunk-major
    (pre-transposed for the PE's lhsT operand), cast to bf16.
  - Device (SPMD, core c owns experts 2c and 2c+1): Z_e = X_e^T.T @ W_e
    as tiled bf16 matmuls with fp32 PSUM accumulation.
      * each k-step's operands (x k-slice + W k-slice) are packed into ONE
        contiguous DRAM chunk -> one DMA per k-step on the sync HWDGE
        ring, delivered in exact PE consumption order; delivery rate
        (~0.85us/chunk) matches PE consumption (~0.86us/k-step) so the
        pipeline never stalls
      * the last k-tile is n-split so the final matmuls' data deps are
        small
      * dummy warm-up matmuls occupy the PE from the tile-body start so
        the HAM clock gate opens (1.2 -> 2.4 GHz) before real data lands
      * PSUM->SBUF copies split across DVE (n0) and ACT (n1) in parallel
      * stores queue on the sync ring BEHIND all loads (FIFO keeps store
        traffic out of the load window); the final store goes on the
        scalar ring so the two tails drain in parallel
  - Host: scatter Z rows back to dispatch pairs, sum top_k contributions,
    add the (gate-weighted) expert biases.
"""

import numpy as np
import ml_dtypes

NUM_EXPERT = 16
D = 1024
TOP_K = 2
T = 2048
N_CORES = 8
EPC = NUM_EXPERT // N_CORES  # experts per core
CAP = 256                    # per-expert dispatch capacity (multiple of 128)
KT = D // 128                # contraction tiles (8)
NT = D // 512                # output free-dim tiles (one PSUM bank each)
MT = CAP // 128              # token tiles (2)

N_DUMMY = 44                 # PE warm-up matmuls (HAM clock-gate)

# Per-expert DRAM image: for k in 0..6: [x_k | W_k] chunks of [128, CAP+D];
# then k=7 split: [x_7 | W_7[:, :512]] and [W_7[:, 512:]].
CK_FULL = CAP + D            # cols of a full k-chunk (1280)
CK_A = CAP + 512             # cols of the k7 first chunk (x + n0 half)
CK_B = 512                   # cols of the k7 second chunk (n1 half)
E_ELEMS = 128 * (7 * CK_FULL + CK_A + CK_B)

TRACE = False                # set by test harness to collect an NTFF profile
LAST_RESULT = None           # BassKernelResults of the most recent run

_NC = None


def _build_nc():
    from concourse import bacc, tile
    import concourse.mybir as mybir

    bf16 = mybir.dt.bfloat16
    f32 = mybir.dt.float32

    nc = bacc.Bacc("TRN2", target_bir_lowering=False, debug=False,
                   num_devices=N_CORES)
    a = nc.declare_dram_parameter("a", [EPC, E_ELEMS], bf16, isOutput=False)
    z = nc.declare_dram_parameter("z", [EPC, CAP, D], bf16, isOutput=True)

    with tile.TileContext(nc, num_cores=N_CORES) as tc:
        with (
            tc.tile_pool(name="wp", bufs=1) as wp,
            tc.tile_pool(name="sp", bufs=1) as sp,
            tc.tile_pool(name="pp", bufs=2, space="PSUM") as pp,
            tc.tile_pool(name="op", bufs=1) as op,
        ):
            # --- PE warm-up: tiny independent matmuls on a scratch tile
            # keep the PE HAM activity monitor busy from the tile-body
            # start so the clock gate opens before real data arrives.
            scr = sp.tile([128, 64], bf16, name="scr", tag="scr")
            nc.gpsimd.memset(scr[:], 0.0)
            # dummy PSUM tile shares tag "ps11" rotation: the dummies and
            # expert-1's ps11 use the same bank (WAW-ordered; e1 starts
            # late so the dummies never delay it).
            psd = pp.tile([128, 512], f32, name="psd", tag="ps11")
            for _ in range(N_DUMMY):
                nc.tensor.matmul(psd[:64, :64], scr[:, :64], scr[:, :64],
                                 start=True, stop=True)

            # --- loads: one DMA per k-step chunk, sync ring, PE order
            cks = {}
            for e in range(EPC):
                base = 0
                for k in range(KT - 1):
                    t_ = wp.tile([128, CK_FULL], bf16,
                                 name=f"c{e}_{k}", tag=f"c{e}_{k}")
                    src = a[e][base:base + 128 * CK_FULL]
                    nc.sync.dma_start(
                        t_[:], src.rearrange("(p f) -> p f", p=128))
                    cks[e, k] = t_
                    base += 128 * CK_FULL
                ta = wp.tile([128, CK_A], bf16, name=f"c{e}_7a",
                             tag=f"c{e}_7a")
                src = a[e][base:base + 128 * CK_A]
                nc.sync.dma_start(ta[:], src.rearrange("(p f) -> p f", p=128))
                base += 128 * CK_A
                tb = wp.tile([128, CK_B], bf16, name=f"c{e}_7b",
                             tag=f"c{e}_7b")
                src = a[e][base:base + 128 * CK_B]
                nc.sync.dma_start(tb[:], src.rearrange("(p f) -> p f", p=128))
                cks[e, KT - 1] = ta
                cks[e, "7b"] = tb

            # --- matmuls, k-outer per expert; 4 (m,n) PSUM banks per
            # expert accumulate in parallel; experts double-buffer banks
            for e in range(EPC):
                pss = {}
                for m in range(MT):
                    for n in range(NT):
                        pss[m, n] = pp.tile([128, 512], f32,
                                            name=f"ps{e}_{m}{n}",
                                            tag=f"ps{m}{n}")
                for k in range(KT):
                    ck = cks[e, k]
                    for n in range(NT):
                        if k == KT - 1 and n == 1:
                            wap = cks[e, "7b"][:, 0:512]
                        else:
                            wap = ck[:, CAP + n * 512:CAP + (n + 1) * 512]
                        for m in range(MT):
                            nc.tensor.matmul(
                                pss[m, n][:],
                                ck[:, m * 128:(m + 1) * 128],
                                wap,
                                start=(k == 0),
                                stop=(k == KT - 1),
                            )
                # copies: n0 on DVE, n1 on ACT (parallel); stores queue on
                # the sync ring behind all loads except the very last one,
                # which goes on the scalar ring so both tails overlap.
                for m in range(MT):
                    ot = op.tile([128, D], bf16, name=f"o{e}_{m}",
                                 tag=f"o{e}_{m}")
                    nc.vector.tensor_copy(ot[:, 0:512], pss[m, 0][:])
                    nc.scalar.copy(ot[:, 512:D], pss[m, 1][:])
                    eng = nc.scalar if (e, m) == (EPC - 1, MT - 1) else nc.sync
                    eng.dma_start(z[e, m * 128:(m + 1) * 128, :], ot[:])
    nc.compile()
    return nc


def _pack_inputs(inp, gi, gs, W):
    """Sort dispatch pairs by expert, gate-fold, pad to CAP, and lay out
    the per-core DRAM image in device chunk order."""
    P = T * TOP_K
    fe = gi.reshape(P)
    fg = gs.reshape(P)
    tok = np.arange(P) // TOP_K

    order = np.argsort(fe, kind="stable")
    counts = np.bincount(fe, minlength=NUM_EXPERT)
    starts = np.zeros(NUM_EXPERT + 1, np.int64)
    np.cumsum(counts, out=starts[1:])
    rank = np.arange(P) - starts[fe[order]]
    ok = rank < CAP
    sel = order[ok]
    rnk = rank[ok]

    xpad = np.zeros((NUM_EXPERT, CAP, D), np.float32)
    xpad[fe[sel], rnk] = inp[tok[sel]] * fg[sel, None]

    # x^T per k-tile: [E, KT, 128p, CAP]
    xk = xpad.reshape(NUM_EXPERT, CAP, KT, 128).transpose(0, 2, 3, 1) \
             .astype(ml_dtypes.bfloat16)
    # W per k-tile: [E, KT, 128p, D]
    wk = W.reshape(NUM_EXPERT, KT, 128, D).astype(ml_dtypes.bfloat16)

    parts = []
    for k in range(KT - 1):
        parts.append(np.concatenate([xk[:, k], wk[:, k]], axis=2)
                     .reshape(NUM_EXPERT, -1))
    parts.append(np.concatenate([xk[:, 7], wk[:, 7, :, 0:512]], axis=2)
                 .reshape(NUM_EXPERT, -1))
    parts.append(np.ascontiguousarray(wk[:, 7, :, 512:D])
                 .reshape(NUM_EXPERT, -1))
    a_dev = np.concatenate(parts, axis=1)
    assert a_dev.shape[1] == E_ELEMS, a_dev.shape
    return a_dev, sel, rnk, order[~ok], fe, tok, fg


def kernel(inp, gate_idx, gate_score, W, b):
    global _NC, LAST_RESULT
    from concourse.bass_utils import run_bass_kernel_spmd

    inp = np.ascontiguousarray(np.asarray(inp, dtype=np.float32))
    gi = np.asarray(gate_idx).astype(np.int64)
    gs = np.asarray(gate_score, dtype=np.float32)
    W = np.asarray(W, dtype=np.float32)
    b = np.asarray(b, dtype=np.float32)

    a_dev, sel, rnk, overflow, fe, tok, fg = _pack_inputs(inp, gi, gs, W)

    if _NC is None:
        _NC = _build_nc()

    in_maps = [
        {"a": a_dev[c * EPC:(c + 1) * EPC]}
        for c in range(N_CORES)
    ]
    res = run_bass_kernel_spmd(_NC, in_maps, list(range(N_CORES)),
                               trace=TRACE)
    LAST_RESULT = res
    zall = np.concatenate(
        [np.asarray(r["z"]).astype(np.float32) for r in res.results],
        axis=0)  # [E,CAP,D]

    P = T * TOP_K
    zpairs = np.zeros((P, D), np.float32)
    zpairs[sel] = zall[fe[sel], rnk]
    # exact f32 fallback for over-capacity pairs (~2% of dispatches)
    if overflow.size:
        fe_o = fe[overflow]
        for e in np.unique(fe_o):
            pi = overflow[fe_o == e]
            zpairs[pi] = (inp[tok[pi]] * fg[pi, None]) @ W[e]

    y = zpairs.reshape(T, TOP_K, D).sum(axis=1)
    y += (gs[:, :, None] * b[gi]).sum(axis=1)
    return y.astype(np.float32)


# revision 11
# speedup vs baseline: 1.1227x; 1.0261x over previous
"""Expert-parallel MoE kernel for Trainium2 (8 NeuronCores).

Strategy (expert-parallel, per sharding hint):
  - Host: sort the T*top_k dispatch pairs by expert, scale each dispatched
    token by its gate score (gate folds into the linear map's input), pad
    each expert's token group to a fixed capacity CAP, lay out chunk-major
    (pre-transposed for the PE's lhsT operand), cast to bf16.
  - Device (SPMD, core c owns experts 2c and 2c+1): Z_e = X_e^T.T @ W_e
    as tiled bf16 matmuls with fp32 PSUM accumulation.
      * each k-step's operands (x k-slice + W k-slice) are packed into ONE
        contiguous DRAM chunk -> one DMA per k-step on the sync HWDGE
        ring, delivered in exact PE consumption order; delivery rate
        (~0.85us/chunk) matches PE consumption (~0.86us/k-step) so the
        pipeline never stalls
      * the last k-tile is n-split so the final matmuls' data deps are
        small
      * dummy warm-up matmuls occupy the PE from the tile-body start so
        the HAM clock gate opens (1.2 -> 2.4 GHz) before real data lands
      * PSUM->SBUF copies split across DVE (n0) and ACT (n1) in parallel
      * stores queue on the sync ring BEHIND all loads (FIFO keeps store
        traffic out of the load window); the final store goes on the
        scalar ring so the two tails drain in parallel
  - Host: scatter Z rows back to dispatch pairs, sum top_k contributions,
    add the (gate-weighted) expert biases.
"""

import numpy as np
import ml_dtypes

NUM_EXPERT = 16
D = 1024
TOP_K = 2
T = 2048
N_CORES = 8
EPC = NUM_EXPERT // N_CORES  # experts per core
CAP = 256                    # per-expert dispatch capacity (multiple of 128)
KT = D // 128                # contraction tiles (8)
NT = D // 512                # output free-dim tiles (one PSUM bank each)
MT = CAP // 128              # token tiles (2)

N_DUMMY = 60                 # PE warm-up matmuls (HAM clock-gate)

# Per-expert DRAM image: k=0 and k=7 are split in two ([x_k | W_k n0-half]
# then [W_k n1-half]) so the pipeline-fill and pipeline-drain data
# dependencies are small; k=1..6 are single [x_k | W_k] chunks.
CK_FULL = CAP + D            # cols of a full k-chunk (1280)
CK_A = CAP + 512             # cols of a split k-chunk's first half
CK_B = 512                   # cols of a split k-chunk's second half
SPLIT_K = (0, KT - 1)
E_ELEMS = 128 * (6 * CK_FULL + 2 * (CK_A + CK_B))

TRACE = False                # set by test harness to collect an NTFF profile
LAST_RESULT = None           # BassKernelResults of the most recent run

_NC = None


def _build_nc():
    from concourse import bacc, tile
    import concourse.mybir as mybir

    bf16 = mybir.dt.bfloat16
    f32 = mybir.dt.float32

    nc = bacc.Bacc("TRN2", target_bir_lowering=False, debug=False,
                   num_devices=N_CORES)
    a = nc.declare_dram_parameter("a", [EPC, E_ELEMS], bf16, isOutput=False)
    z = nc.declare_dram_parameter("z", [EPC, CAP, D], bf16, isOutput=True)

    with tile.TileContext(nc, num_cores=N_CORES) as tc:
        with (
            tc.tile_pool(name="wp", bufs=1) as wp,
            tc.tile_pool(name="sp", bufs=1) as sp,
            tc.tile_pool(name="pp", bufs=2, space="PSUM") as pp,
            tc.tile_pool(name="op", bufs=1) as op,
        ):
            # --- PE warm-up: tiny independent matmuls on a scratch tile
            # keep the PE HAM activity monitor busy from the tile-body
            # start so the clock gate opens before real data arrives.
            scr = sp.tile([128, 64], bf16, name="scr", tag="scr")
            nc.gpsimd.memset(scr[:], 0.0)
            # dummy PSUM tile shares tag "ps11" rotation: the dummies and
            # expert-1's ps11 use the same bank (WAW-ordered; e1 starts
            # late so the dummies never delay it).
            psd = pp.tile([128, 512], f32, name="psd", tag="ps11")
            for _ in range(N_DUMMY):
                nc.tensor.matmul(psd[:64, :64], scr[:, :64], scr[:, :64],
                                 start=True, stop=True)

            # --- loads: one DMA per k-step chunk, sync ring, PE order
            cks = {}
            for e in range(EPC):
                base = 0
                for k in range(KT):
                    if k in SPLIT_K:
                        ta = wp.tile([128, CK_A], bf16, name=f"c{e}_{k}a",
                                     tag=f"c{e}_{k}a")
                        src = a[e][base:base + 128 * CK_A]
                        nc.sync.dma_start(
                            ta[:], src.rearrange("(p f) -> p f", p=128))
                        base += 128 * CK_A
                        tb = wp.tile([128, CK_B], bf16, name=f"c{e}_{k}b",
                                     tag=f"c{e}_{k}b")
                        src = a[e][base:base + 128 * CK_B]
                        nc.sync.dma_start(
                            tb[:], src.rearrange("(p f) -> p f", p=128))
                        base += 128 * CK_B
                        cks[e, k] = ta
                        cks[e, k, "b"] = tb
                    else:
                        t_ = wp.tile([128, CK_FULL], bf16,
                                     name=f"c{e}_{k}", tag=f"c{e}_{k}")
                        src = a[e][base:base + 128 * CK_FULL]
                        nc.sync.dma_start(
                            t_[:], src.rearrange("(p f) -> p f", p=128))
                        cks[e, k] = t_
                        base += 128 * CK_FULL

            # --- matmuls, k-outer per expert; 4 (m,n) PSUM banks per
            # expert accumulate in parallel; experts double-buffer banks
            for e in range(EPC):
                pss = {}
                for m in range(MT):
                    for n in range(NT):
                        pss[m, n] = pp.tile([128, 512], f32,
                                            name=f"ps{e}_{m}{n}",
                                            tag=f"ps{m}{n}")
                for k in range(KT):
                    ck = cks[e, k]
                    for n in range(NT):
                        if k in SPLIT_K and n == 1:
                            wap = cks[e, k, "b"][:, 0:512]
                        else:
                            wap = ck[:, CAP + n * 512:CAP + (n + 1) * 512]
                        for m in range(MT):
                            nc.tensor.matmul(
                                pss[m, n][:],
                                ck[:, m * 128:(m + 1) * 128],
                                wap,
                                start=(k == 0),
                                stop=(k == KT - 1),
                            )
                # copies: n0 on DVE, n1 on ACT (parallel); stores queue on
                # the sync ring behind all loads except the very last
                # m-tile, whose halves drain on both rings in parallel.
                for m in range(MT):
                    ot = op.tile([128, D], bf16, name=f"o{e}_{m}",
                                 tag=f"o{e}_{m}")
                    nc.vector.tensor_copy(ot[:, 0:512], pss[m, 0][:])
                    nc.scalar.copy(ot[:, 512:D], pss[m, 1][:])
                    zrow = z[e, m * 128:(m + 1) * 128, :]
                    if (e, m) == (EPC - 1, MT - 1):
                        nc.scalar.dma_start(zrow[:, 0:512], ot[:, 0:512])
                        nc.sync.dma_start(zrow[:, 512:D], ot[:, 512:D])
                    else:
                        nc.sync.dma_start(zrow, ot[:])
    nc.compile()
    return nc


def _pack_inputs(inp, gi, gs, W):
    """Sort dispatch pairs by expert, gate-fold, pad to CAP, and lay out
    the per-core DRAM image in device chunk order."""
    P = T * TOP_K
    fe = gi.reshape(P)
    fg = gs.reshape(P)
    tok = np.arange(P) // TOP_K

    order = np.argsort(fe, kind="stable")
    counts = np.bincount(fe, minlength=NUM_EXPERT)
    starts = np.zeros(NUM_EXPERT + 1, np.int64)
    np.cumsum(counts, out=starts[1:])
    rank = np.arange(P) - starts[fe[order]]
    ok = rank < CAP
    sel = order[ok]
    rnk = rank[ok]

    xpad = np.zeros((NUM_EXPERT, CAP, D), np.float32)
    xpad[fe[sel], rnk] = inp[tok[sel]] * fg[sel, None]

    # x^T per k-tile: [E, KT, 128p, CAP]
    xk = xpad.reshape(NUM_EXPERT, CAP, KT, 128).transpose(0, 2, 3, 1) \
             .astype(ml_dtypes.bfloat16)
    # W per k-tile: [E, KT, 128p, D]
    wk = W.reshape(NUM_EXPERT, KT, 128, D).astype(ml_dtypes.bfloat16)

    parts = []
    for k in range(KT):
        if k in SPLIT_K:
            parts.append(np.concatenate([xk[:, k], wk[:, k, :, 0:512]],
                                        axis=2).reshape(NUM_EXPERT, -1))
            parts.append(np.ascontiguousarray(wk[:, k, :, 512:D])
                         .reshape(NUM_EXPERT, -1))
        else:
            parts.append(np.concatenate([xk[:, k], wk[:, k]], axis=2)
                         .reshape(NUM_EXPERT, -1))
    a_dev = np.concatenate(parts, axis=1)
    assert a_dev.shape[1] == E_ELEMS, a_dev.shape
    return a_dev, sel, rnk, order[~ok], fe, tok, fg


def kernel(inp, gate_idx, gate_score, W, b):
    global _NC, LAST_RESULT
    from concourse.bass_utils import run_bass_kernel_spmd

    inp = np.ascontiguousarray(np.asarray(inp, dtype=np.float32))
    gi = np.asarray(gate_idx).astype(np.int64)
    gs = np.asarray(gate_score, dtype=np.float32)
    W = np.asarray(W, dtype=np.float32)
    b = np.asarray(b, dtype=np.float32)

    a_dev, sel, rnk, overflow, fe, tok, fg = _pack_inputs(inp, gi, gs, W)

    if _NC is None:
        _NC = _build_nc()

    in_maps = [
        {"a": a_dev[c * EPC:(c + 1) * EPC]}
        for c in range(N_CORES)
    ]
    res = run_bass_kernel_spmd(_NC, in_maps, list(range(N_CORES)),
                               trace=TRACE)
    LAST_RESULT = res
    zall = np.concatenate(
        [np.asarray(r["z"]).astype(np.float32) for r in res.results],
        axis=0)  # [E,CAP,D]

    P = T * TOP_K
    zpairs = np.zeros((P, D), np.float32)
    zpairs[sel] = zall[fe[sel], rnk]
    # exact f32 fallback for over-capacity pairs (~2% of dispatches)
    if overflow.size:
        fe_o = fe[overflow]
        for e in np.unique(fe_o):
            pi = overflow[fe_o == e]
            zpairs[pi] = (inp[tok[pi]] * fg[pi, None]) @ W[e]

    y = zpairs.reshape(T, TOP_K, D).sum(axis=1)
    y += (gs[:, :, None] * b[gi]).sum(axis=1)
    return y.astype(np.float32)
